# revision 10
# baseline (speedup 1.0000x reference)
"""Trainium2 Bass kernel for nn_Aligner (location-sensitive attention + GRU scan).

Sharding: data-parallel over batch across 8 NeuronCores (4 utterances/core),
weights replicated; each core runs the full sequential T-step scan.

V2 design notes (per core, BL=4):
 - All recurrent contractions (gh, gprev, qp, ctx, t1) run as 4x col-tiled
   PE rounds: 4 K-chunks execute CONCURRENTLY in col-groups {0,32,64,96}
   (tile_position), partials land in one PSUM bank, get copied to SBUF bf16
   and collapsed with a single [128,4] selector matmul. Stationaries are
   zero-padded to M=32 so every partial row is initialized.
 - score = weff*ash + key + [qp|bk]*[bsel|ones] entirely in PE (key added
   via an identity matmul; bk rides as a 5th lhsT row); tanh is ACT-only
   PSUM->SBUF bf16. No DVE in the score path.
 - energy lands [1, (b,s)] in PSUM, is copied to SBUF by DVE+ACT halves and
   partition-scattered to [4, S] with ONE SBUF->SBUF DMA (no DRAM bounce).
 - gates: z-gate preactivations are emitted LAST (needed last), r first;
   mix/m1 softmax mixing factors precompute at iteration start.
 - frame @ w_t1.T + b_t1 and q @ w_ih.T + bias are both precomputed for all
   steps into gq rows [R, 4096]; the loop is unrolled 2x so gq rows prefetch
   one step ahead into alternating buffers.
 - conv1d(align) via composed [C2,31] weight and an overlapping-window DMA
   from a zero-padded DRAM buffer (bf16).
"""

import sys
import numpy as np

sys.path.insert(0, "/opt/trn_rl_repo")

import bass_rust
from concourse import bass, bacc, tile
import concourse.mybir as mybir
from concourse.bass_utils import run_bass_kernel_spmd

F32 = mybir.dt.float32
BF16 = mybir.dt.bfloat16
AF = mybir.ActivationFunctionType
ALU = mybir.AluOpType
PE = mybir.EngineType.PE

B_FULL, S, T_FULL = 32, 256, 800
I, H, M, C, LOC, KC = 512, 256, 80, 1024, 32, 31
C2 = C // 2
NCORES = 8
BL = B_FULL // NCORES          # 4
G3 = 3 * C                     # 3072
G4 = G3 + C                    # 4096 (gq row: gates + t1-frame part)
PAD = S + 30                   # 286
COLTILE = False


def _ap(handle_or_ap, steps_counts, offset=0):
    """Raw [step,count] access pattern over a tensor (element units)."""
    if isinstance(handle_or_ap, bass_rust.AP):
        ap = handle_or_ap.copy()
    else:
        ap = handle_or_ap.ap().copy()
    ap.ap = bass_rust.VecI64Pair(steps_counts)
    ap.offset = offset
    return ap


def build_program(T):
    assert T % 2 == 0, "loop is unrolled 2x"
    nc = bacc.Bacc("TRN2", target_bir_lowering=False, debug=False)
    R = BL * T

    def din(name, shape, dt):
        return nc.dram_tensor(name, list(shape), dt, kind="ExternalInput")

    enc_bf_d = din("enc_bf", [128, 8, I], BF16)
    encT_d = din("encT", [128, 4, BL * S], F32)
    qT_d = din("qT", [128, 2, R], BF16)
    frTT_d = din("frTT", [M + 1, R], BF16)
    wihp_d = din("wihp", [128, 4, G3], BF16)
    whh_d = din("whh", [128, 8, G3], BF16)
    wq_d = din("wq", [128, 8, C2], BF16)
    wt1a_d = din("wt1a", [128, 4, C], BF16)
    wt1h_d = din("wt1h", [128, 8, C], BF16)
    wt1fb_d = din("wt1fb", [M + 1, C], BF16)
    wihq_d = din("wihq", [128, 2, G3], BF16)
    wk_d = din("wk", [128, 4, C2], F32)
    weff_d = din("weff", [KC, C2], F32)
    wagg_d = din("wagg", [128, 4], BF16)
    bias1_d = din("bias1", [1, G3], F32)
    bhhn_d = din("bhhn", [1, C], F32)
    bselo_d = din("bselo", [5, BL * S], F32)
    bkr_d = din("bkr", [1, C2], F32)
    i4bf_d = din("i4bf", [BL, BL], BF16)
    i128_d = din("i128", [128, 128], BF16)
    sel4_d = din("sel4", [128, BL], BF16)
    id4_d = din("id4", [BL, BL], F32)
    ones1_d = din("ones1", [1, 128], F32)
    wt2r_d = din("wt2r", [BL, C], F32)

    alphas_d = nc.dram_tensor("alphas", [R, S], F32, kind="ExternalOutput")

    gq_d = nc.dram_tensor("gq_scratch", [R + BL, G4], BF16)
    eb_d = nc.dram_tensor("e_bounce", [BL * S], F32)
    apd = nc.dram_tensor("align_pad", [BL, PAD], F32)

    with tile.TileContext(nc) as tc:
        with (
            tc.tile_pool(name="const", bufs=1) as cpool,
            tc.tile_pool(name="state", bufs=1) as spool,
            tc.tile_pool(name="work", bufs=1) as wpool,
            tc.tile_pool(name="psum", bufs=1, space="PSUM") as ppool,
        ):
            def load(dram, shape, dt, tag):
                t = cpool.tile(list(shape), dt, tag=tag)
                nc.sync.dma_start(t[:], dram.ap())
                return t

            ones1 = load(ones1_d, [1, 128], F32, "ones1")
            wt1fb = load(wt1fb_d, [M + 1, C], BF16, "wt1fb")
            key_sb = cpool.tile([128, 4, BL * S], BF16, tag="key_sb")

            # ===== precompute (aliased into const slots, loaded later) =====
            if True:
                encT = cpool.tile([128, 4, BL * S], F32, tag="whh")
                nc.sync.dma_start(encT[:], encT_d.ap())
                wk = cpool.tile([128, 4, C2], F32, tag="wq")
                nc.sync.dma_start(wk[:], wk_d.ap())
                wihq = cpool.tile([128, 2, G3], BF16, tag="wihp")
                nc.sync.dma_start(wihq[:], wihq_d.ap())
                bias1 = cpool.tile([1, G3], F32, tag="wt1h")
                nc.sync.dma_start(bias1[:], bias1_d.ap())

                # key[c2chunk, (b,s)] = w_k.T^T @ encT   (fp32 -> bf16)
                for mc in range(4):
                    for nk in range(2):
                        kps = ppool.tile([128, 512], F32, tag="pS", bufs=2)
                        for kc in range(4):
                            nc.tensor.matmul(
                                kps[:],
                                wk[:, kc, mc * 128:(mc + 1) * 128],
                                encT[:, kc, nk * 512:(nk + 1) * 512],
                                start=(kc == 0), stop=(kc == 3))
                        if nk == 0:
                            nc.vector.tensor_copy(
                                key_sb[:, mc, nk * 512:(nk + 1) * 512], kps[:])
                        else:
                            nc.scalar.copy(
                                key_sb[:, mc, nk * 512:(nk + 1) * 512], kps[:])

                # gq rows: [q @ w_ih_q.T + bias1 | fr @ w_t1f.T + b_t1]
                nmc = (R + 127) // 128
                for mc in range(nmc):
                    r0 = mc * 128
                    rr = min(128, R - r0)
                    qts = cpool.tile([128, 2, 128], BF16, tag="enc_bf", bufs=1)
                    nc.sync.dma_start(qts[:, :, :rr],
                                      qT_d.ap()[:, :, r0:r0 + rr])
                    frt = cpool.tile([M + 1, 128], BF16, tag="frt", bufs=1)
                    nc.sync.dma_start(frt[:, :rr], frTT_d.ap()[:, r0:r0 + rr])
                    for nkg in range(8):
                        col = nkg * 512
                        gps = ppool.tile([128, 512], F32, tag="pS", bufs=2)
                        if nkg < 6:
                            for kc in range(2):
                                nc.tensor.matmul(
                                    gps[:rr, :], qts[:, kc, :rr],
                                    wihq[:, kc, col:col + 512],
                                    start=(kc == 0), stop=False)
                            nc.tensor.matmul(
                                gps[:rr, :], ones1[:, :rr],
                                bias1[:, col:col + 512],
                                start=False, stop=True)
                        else:
                            nc.tensor.matmul(
                                gps[:rr, :], frt[:, :rr],
                                wt1fb[:, (nkg - 6) * 512:(nkg - 5) * 512],
                                start=True, stop=True)
                        gsb = cpool.tile([128, 512], BF16, tag="wt1a", bufs=1)
                        if nkg % 2 == 0:
                            nc.vector.tensor_copy(gsb[:rr, :], gps[:rr, :])
                        else:
                            nc.scalar.copy(gsb[:rr, :], gps[:rr, :])
                        nc.sync.dma_start(
                            gq_d.ap()[r0:r0 + rr, col:col + 512], gsb[:rr, :])
                # zero-pad prefetch overrun rows
                gz = cpool.tile([128, 512], BF16, tag="wt1a", bufs=1)
                nc.gpsimd.memset(gz[0:BL, :], 0.0)
                for nkg in range(8):
                    nc.sync.dma_start(
                        gq_d.ap()[R:R + BL, nkg * 512:(nkg + 1) * 512],
                        gz[0:BL, :])

            # big constants loaded after the precompute pool is done
            enc_bf = load(enc_bf_d, [128, 8, I], BF16, "enc_bf")
            wihp = load(wihp_d, [128, 4, G3], BF16, "wihp")
            whh = load(whh_d, [128, 8, G3], BF16, "whh")
            wq = load(wq_d, [128, 8, C2], BF16, "wq")
            wt1a = load(wt1a_d, [128, 4, C], BF16, "wt1a")
            wt1h = load(wt1h_d, [128, 8, C], BF16, "wt1h")
            weff = load(weff_d, [KC, C2], F32, "weff")
            wagg = load(wagg_d, [128, 4], BF16, "wagg")
            bhhn = load(bhhn_d, [1, C], F32, "bhhn")
            bselo = load(bselo_d, [5, BL * S], F32, "bselo")
            i4bf = load(i4bf_d, [BL, BL], BF16, "i4bf")
            i128 = load(i128_d, [128, 128], BF16, "i128")
            sel4 = load(sel4_d, [128, BL], BF16, "sel4")
            id4 = load(id4_d, [BL, BL], F32, "id4")
            wt2r = load(wt2r_d, [BL, C], F32, "wt2r")

            # ================= state =================
            hrow = spool.tile([BL, C], F32)
            hTp = spool.tile([128, 8, 32], BF16)
            ctxTp = spool.tile([128, 4, 32], BF16)
            aD = spool.tile([128, 8, 32], BF16)
            alf = spool.tile([BL, S + 1], F32)
            trans = spool.tile([BL, 1], F32)
            qp5 = spool.tile([5, C2], F32)
            ash = spool.tile([KC, BL * S], F32)
            alsc = spool.tile([BL, S], F32)
            sg0 = spool.tile([128, 512], BF16)
            sg1 = spool.tile([128, 512], BF16)
            gqA = spool.tile([BL, G4], BF16)
            gqB = spool.tile([BL, G4], BF16)

            nc.gpsimd.memset(hrow[:], 0.0)
            nc.gpsimd.memset(hTp[:], 0.0)
            nc.gpsimd.memset(ctxTp[:], 0.0)
            nc.gpsimd.memset(aD[:], 0.0)
            nc.gpsimd.memset(alf[:], 0.0)
            nc.gpsimd.memset(alf[:, 1:2], 1.0)
            nc.gpsimd.memset(trans[:], 0.5)
            nc.gpsimd.memset(sg0[:], 0.0)
            nc.gpsimd.memset(sg1[:], 0.0)
            for b in range(BL):
                nc.gpsimd.memset(aD[0:1, 2 * b, b:b + 1], 1.0)
            nc.sync.dma_start(qp5[4:5, :], bkr_d.ap())
            # initial align = 1/S -> padded conv window buffer
            nc.gpsimd.memset(ash[:], 0.0)
            nc.sync.dma_start(apd.ap()[:, :], ash[0:BL, 0:PAD])
            nc.gpsimd.memset(alsc[:], 1.0 / S)
            nc.sync.dma_start(apd.ap()[:, 15:15 + S], alsc[:])
            nc.sync.dma_start(ash[:, :],
                              _ap(apd, [[1, KC], [PAD, BL], [1, S]]))
            nc.sync.dma_start(gqA[:], gq_d.ap()[0:BL, :])

            sg_state = [0]

            def contract512(out_ps, terms, extras, n_start=True):
                """out_ps [4,512] (PSUM) = sum of lhsT.T @ rhs terms.

                terms: list of (tile, chunk, rhs_ap) with tile [128, nch, 32]
                (cols 4..31 zero).  extras: small-K (lhsT_ap, rhs_ap) added
                directly at the end.
                """
                if COLTILE and len(terms) >= 4:
                    assert len(terms) % 4 == 0
                    pg = ppool.tile([128, 512], F32, tag="pP", bufs=2)
                    for i, (tl, kc, rhs) in enumerate(terms):
                        c = i % 4
                        nc.tensor.matmul(
                            pg[32 * c:32 * c + 32, :], tl[:, kc, :], rhs,
                            start=(i < 4), stop=(i >= len(terms) - 4),
                            tile_position=(0, 32 * c), skip_group_check=True)
                    sg = sg0 if sg_state[0] == 0 else sg1
                    sg_state[0] ^= 1
                    nc.vector.tensor_copy(sg[:], pg[:])
                    nc.tensor.matmul(out_ps, sel4[:], sg[:],
                                     start=n_start, stop=(not extras))
                else:
                    for i, (tl, kc, rhs) in enumerate(terms):
                        nc.tensor.matmul(
                            out_ps, tl[:, kc, 0:BL], rhs,
                            start=(n_start and i == 0),
                            stop=(not extras and i == len(terms) - 1))
                for j, (lt, rhs) in enumerate(extras):
                    nc.tensor.matmul(out_ps, lt, rhs,
                                     start=False, stop=(j == len(extras) - 1))
                return out_ps

            def ctx_block():
                cps = ppool.tile([BL, I], F32, tag="pG", bufs=2)
                contract512(
                    cps[:],
                    [(aD, kc, enc_bf[:, kc, :]) for kc in range(8)], [])
                ctx_b = wpool.tile([BL, I], F32, tag="ctxb")
                nc.vector.tensor_copy(ctx_b[:], cps[:])
                ctps = ppool.tile([128, 4, BL], F32, tag="pG", bufs=2)
                for ck in range(4):
                    nc.tensor.transpose(
                        ctps[:, ck, :], ctx_b[:, ck * 128:(ck + 1) * 128],
                        id4[:])
                nc.vector.tensor_copy(
                    _ap(ctxTp[:], [[128, 128], [32, 4], [1, 4]]), ctps[:])

            ctx_block()

            def step(iv_lo, gq_use):
                """One scan step; iv_lo = dram row start (dyn slice)."""
                # ---- softmax mixing factors (uses prev alf/trans) ----
                omt = wpool.tile([BL, 1], F32, tag="omt", bufs=1)
                nc.vector.tensor_scalar(omt[:], trans[:], -1.0, 1.0,
                                        ALU.mult, ALU.add)
                m1 = wpool.tile([BL, S], F32, tag="al", bufs=3)
                nc.vector.tensor_scalar(m1[:], alf[:, 1:], omt[:], 1e-7,
                                        ALU.mult, ALU.add)
                mix = wpool.tile([BL, S], F32, tag="al", bufs=3)
                nc.vector.scalar_tensor_tensor(
                    mix[:], alf[:, 0:S], trans[:], m1[:], ALU.mult, ALU.add)

                # ---- gate preactivations (r, ghn, xn first; z LAST) ----
                def gh_terms(nk):
                    return [(hTp, kc, whh[:, kc, nk * 512:(nk + 1) * 512])
                            for kc in range(8)]

                def gp_terms(nk):
                    return [(ctxTp, kc, wihp[:, kc, nk * 512:(nk + 1) * 512])
                            for kc in range(4)]

                def gq_extra(nk):
                    return (i4bf[:], gq_use[:, nk * 512:(nk + 1) * 512])

                # ghn (gh cols 2C..3C) + b_hh_n
                hn05 = wpool.tile([BL, C], F32, tag="gnt", bufs=1)
                for e in range(2):
                    nk = 4 + e
                    ps = ppool.tile([BL, 512], F32, tag="pG", bufs=2)
                    contract512(ps[:], gh_terms(nk),
                                [(ones1[:, 0:BL], bhhn[:, e * 512:(e + 1) * 512])])
                    nc.scalar.activation(hn05[:, e * 512:(e + 1) * 512],
                                         ps[:], AF.Copy, scale=0.5)
                # r (cols 0..C)
                trzr = wpool.tile([BL, C], F32, tag="gact", bufs=2)
                for nk in range(2):
                    ps = ppool.tile([BL, 512], F32, tag="pG", bufs=2)
                    contract512(ps[:], gh_terms(nk) + gp_terms(nk),
                                [gq_extra(nk)])
                    nc.scalar.activation(trzr[:, nk * 512:(nk + 1) * 512],
                                         ps[:], AF.Tanh, scale=0.5)
                o2 = wpool.tile([BL, C], F32, tag="gtmp", bufs=2)
                nc.vector.scalar_tensor_tensor(
                    o2[:], trzr[:], 1.0, hn05[:], ALU.add, ALU.mult)
                # xn (gx cols 2C..3C)
                narg = wpool.tile([BL, C], F32, tag="gnt", bufs=1)
                for e in range(2):
                    nk = 4 + e
                    ps = ppool.tile([BL, 512], F32, tag="pG", bufs=2)
                    contract512(ps[:], gp_terms(nk), [gq_extra(nk)])
                    nc.vector.tensor_add(narg[:, e * 512:(e + 1) * 512],
                                         ps[:], o2[:, e * 512:(e + 1) * 512])
                ngate = wpool.tile([BL, C], F32, tag="ngate")
                nc.scalar.activation(ngate[:], narg[:], AF.Tanh)
                # z (cols C..2C)
                tzr = wpool.tile([BL, C], F32, tag="gact", bufs=2)
                for e in range(2):
                    nk = 2 + e
                    ps = ppool.tile([BL, 512], F32, tag="pG", bufs=2)
                    contract512(ps[:], gh_terms(nk) + gp_terms(nk),
                                [gq_extra(nk)])
                    nc.scalar.activation(tzr[:, e * 512:(e + 1) * 512],
                                         ps[:], AF.Tanh, scale=0.5)
                dmn = wpool.tile([BL, C], F32, tag="gtmp", bufs=2)
                nc.vector.tensor_sub(dmn[:], hrow[:], ngate[:])
                o5 = wpool.tile([BL, C], F32, tag="gtmp", bufs=2)
                nc.vector.scalar_tensor_tensor(
                    o5[:], tzr[:], 1.0, dmn[:], ALU.add, ALU.mult)
                nc.vector.scalar_tensor_tensor(
                    hrow[:], o5[:], 0.5, ngate[:], ALU.mult, ALU.add)

                # ---- hT ----
                tps = ppool.tile([128, 8, BL], F32, tag="pG", bufs=2)
                for kc in range(8):
                    nc.tensor.transpose(
                        tps[:, kc, :], hrow[:, kc * 128:(kc + 1) * 128], id4[:])
                nc.vector.tensor_copy(
                    _ap(hTp[:], [[256, 128], [32, 8], [1, 4]]), tps[:])

                # ---- qp = h_new @ w_q.T (+ bk row persists) ----
                qps = ppool.tile([BL, C2], F32, tag="pG", bufs=2)
                contract512(qps[:],
                            [(hTp, kc, wq[:, kc, :]) for kc in range(8)], [])
                nc.vector.tensor_copy(qp5[0:BL, :], qps[:])

                # ---- score + tanh + energy ----
                eps = ppool.tile([1, BL * S], F32, tag="pE", bufs=1)
                for mc in range(4):
                    taut = wpool.tile([128, BL * S], BF16, tag="taut", bufs=2)
                    for nk in range(2):
                        cs = slice(nk * 512, (nk + 1) * 512)
                        scps = ppool.tile([128, 512], F32, tag="pS", bufs=2)
                        nc.tensor.matmul(
                            scps[:], weff[:, mc * 128:(mc + 1) * 128],
                            ash[:, cs], start=True, stop=False)
                        nc.tensor.matmul(
                            scps[:], i128[:], key_sb[:, mc, cs],
                            start=False, stop=False)
                        nc.tensor.matmul(
                            scps[:], qp5[:, mc * 128:(mc + 1) * 128],
                            bselo[:, cs], start=False, stop=True)
                        nc.scalar.activation(taut[:, cs], scps[:], AF.Tanh)
                        nc.tensor.matmul(
                            eps[:, cs], wagg[:, mc:mc + 1], taut[:, cs],
                            start=(mc == 0), stop=(mc == 3))

                # ---- energy -> [4, S] via SBUF scatter DMA; softmax ----
                erow = wpool.tile([1, BL * S], F32, tag="erow")
                nc.vector.tensor_copy(erow[:, 0:512], eps[:, 0:512])
                nc.scalar.copy(erow[:, 512:1024], eps[:, 512:1024])
                nc.sync.dma_start(eb_d.ap()[:], erow[:])
                e4r = wpool.tile([BL, S], F32, tag="e4r")
                nc.sync.dma_start(e4r[:], _ap(eb_d, [[S, BL], [1, S]]))
                e4 = wpool.tile([BL, S], F32, tag="e4")
                nc.scalar.activation(e4[:], e4r[:], AF.Exp)

                # alpha recursion
                u = wpool.tile([BL, S], F32, tag="al", bufs=3)
                nc.vector.tensor_mul(u[:], mix[:], e4[:])
                usum = wpool.tile([BL, 1], F32, tag="usum", bufs=2)
                nc.vector.reduce_sum(usum[:], u[:], mybir.AxisListType.X)
                urec = wpool.tile([BL, 1], F32, tag="urec", bufs=2)
                nc.vector.reciprocal(urec[:], usum[:])
                nc.vector.tensor_scalar(alf[:, 1:], u[:], urec[:], None,
                                        ALU.mult)
                nc.sync.dma_start(alphas_d.ap()[iv_lo, :], alf[:, 1:])

                # align (= softmax(energy)) for next step's conv
                zs = wpool.tile([BL, 1], F32, tag="zs", bufs=1)
                nc.vector.reduce_sum(zs[:], e4[:], mybir.AxisListType.X)
                zr = wpool.tile([BL, 1], F32, tag="zr", bufs=2)
                nc.vector.reciprocal(zr[:], zs[:])
                nc.vector.tensor_scalar(alsc[:], e4[:], zr[:], None, ALU.mult)
                nc.sync.dma_start(apd.ap()[:, 15:15 + S], alsc[:])
                nc.sync.dma_start(ash[:, :],
                                  _ap(apd, [[1, KC], [PAD, BL], [1, S]]))

                # ---- alpha -> aD (block diagonal, bf16) ----
                aps = ppool.tile([128, 2, BL], F32, tag="pG", bufs=2)
                nc.tensor.transpose(aps[:, 0, :], alf[:, 1:129], id4[:])
                nc.tensor.transpose(aps[:, 1, :], alf[:, 129:257], id4[:])
                for seg in range(2):
                    nc.vector.tensor_copy(
                        _ap(aD[:], [[256, 128], [65, BL]], 32 * seg),
                        aps[:, seg, :])

                # ---- ctx (= attend_t = prev_{t+1}) ----
                ctx_block()

                # ---- t-branch: trans_{t+1} ----
                tt1 = wpool.tile([BL, C], F32, tag="gact", bufs=2)
                for nk in range(2):
                    cs = slice(nk * 512, (nk + 1) * 512)
                    t1p = ppool.tile([BL, 512], F32, tag="pG", bufs=2)
                    contract512(
                        t1p[:],
                        [(hTp, kc, wt1h[:, kc, cs]) for kc in range(8)]
                        + [(ctxTp, kc, wt1a[:, kc, cs]) for kc in range(4)],
                        [(i4bf[:], gq_use[:, G3 + nk * 512:G3 + (nk + 1) * 512])])
                    nc.scalar.activation(tt1[:, cs], t1p[:], AF.Tanh)
                tu = wpool.tile([BL, C], F32, tag="gtmp", bufs=2)
                nc.vector.tensor_mul(tu[:], tt1[:], wt2r[:])
                ts = wpool.tile([BL, 1], F32, tag="ts", bufs=2)
                nc.vector.reduce_sum(ts[:], tu[:], mybir.AxisListType.X)
                tt = wpool.tile([BL, 1], F32, tag="tt", bufs=2)
                nc.scalar.activation(tt[:], ts[:], AF.Tanh, scale=0.5)
                nc.vector.tensor_scalar(trans[:], tt[:], 0.5, 0.5,
                                        ALU.mult, ALU.add)

            # ================= scan (2x unrolled) =================
            with tc.For_i(0, R, 2 * BL, hint_engines=(PE,),
                          staggered_reset=True) as iv:
                nc.sync.dma_start(gqB[:], gq_d.ap()[bass.ds(iv + BL, BL), :])
                step(bass.ds(iv, BL), gqA)
                nc.sync.dma_start(gqA[:],
                                  gq_d.ap()[bass.ds(iv + 2 * BL, BL), :])
                step(bass.ds(iv + BL, BL), gqB)

    return nc


def _prep_shared(inputs):
    w_ih = np.asarray(inputs["w_ih"], np.float32)
    w_hh = np.asarray(inputs["w_hh"], np.float32)
    b_ih = np.asarray(inputs["b_ih"], np.float32)
    b_hh = np.asarray(inputs["b_hh"], np.float32)
    w_q = np.asarray(inputs["w_q"], np.float32)
    w_loc1 = np.asarray(inputs["w_loc1"], np.float32)
    w_loc2 = np.asarray(inputs["w_loc2"], np.float32)
    w_k = np.asarray(inputs["w_k"], np.float32)
    b_k = np.asarray(inputs["b_k"], np.float32)
    w_agg = np.asarray(inputs["w_agg"], np.float32)
    w_t1 = np.asarray(inputs["w_t1"], np.float32)
    b_t1 = np.asarray(inputs["b_t1"], np.float32)
    w_t2 = np.asarray(inputs["w_t2"], np.float32)

    w_eff = w_loc2 @ w_loc1[:, 0, :]  # [C2, KC]
    bias1 = b_ih + np.concatenate([b_hh[:2 * C], np.zeros(C, np.float32)])
    bselo = np.zeros((5, BL * S), np.float32)
    for b in range(BL):
        bselo[b, b * S:(b + 1) * S] = 1.0
    bselo[4, :] = 1.0
    sel4 = np.zeros((128, BL), np.float32)
    for c in range(4):
        for b in range(BL):
            sel4[32 * c + b, b] = 1.0
    wt1fb = np.concatenate([w_t1[:, I:I + M].T, b_t1.reshape(1, C)], axis=0)

    cc = np.ascontiguousarray

    def chunk(a):  # [nk*128, X] -> [128, nk, X]
        nk = a.shape[0] // 128
        return cc(a.reshape(nk, 128, -1).transpose(1, 0, 2))

    return {
        "wihp": chunk(w_ih[:, H:].T),
        "whh": chunk(w_hh.T),
        "wq": chunk(w_q.T),
        "wt1a": chunk(w_t1[:, :I].T),
        "wt1h": chunk(w_t1[:, I + M:].T),
        "wt1fb": cc(wt1fb),
        "wihq": chunk(w_ih[:, :H].T),
        "wk": chunk(w_k.T),
        "weff": cc(w_eff.T),
        "wagg": cc(w_agg.reshape(4, 128).T),
        "bkr": b_k.reshape(1, C2),
        "bias1": bias1.reshape(1, G3),
        "bhhn": cc(b_hh[2 * C:].reshape(1, C)),
        "bselo": bselo,
        "i4bf": np.eye(BL, dtype=np.float32),
        "i128": np.eye(128, dtype=np.float32),
        "sel4": sel4,
        "id4": np.eye(BL, dtype=np.float32),
        "ones1": np.ones((1, 128), np.float32),
        "wt2r": np.tile(w_t2.reshape(1, C), (BL, 1)),
    }


_BF16_NAMES = {"enc_bf", "qT", "frTT", "wihp", "whh", "wq", "wt1a", "wt1h",
               "wt1fb", "wihq", "wagg", "i4bf", "i128", "sel4"}


def make_in_maps(inputs):
    import ml_dtypes

    def cast(name, arr):
        if name in _BF16_NAMES:
            return np.asarray(arr, np.float32).astype(ml_dtypes.bfloat16)
        return np.ascontiguousarray(arr, np.float32)

    T = inputs["queries"].shape[1]
    shared = _prep_shared(inputs)
    enc = np.asarray(inputs["encodings"], np.float32)
    qs = np.asarray(inputs["queries"], np.float32)
    outs = np.asarray(inputs["outputs"], np.float32)

    in_maps = []
    for c in range(NCORES):
        sl = slice(c * BL, (c + 1) * BL)
        e = enc[sl].reshape(BL * S, I)
        q = qs[sl]
        fr = outs[sl]  # [BL, T, M]
        m = {k: cast(k, v) for k, v in shared.items()}
        m["enc_bf"] = cast("enc_bf", e.reshape(8, 128, I).transpose(1, 0, 2))
        m["encT"] = cast("encT", e.T.reshape(4, 128, BL * S).transpose(1, 0, 2))
        m["qT"] = cast("qT", q.transpose(2, 1, 0).reshape(
            2, 128, T * BL).transpose(1, 0, 2))
        frTT = np.concatenate(
            [fr.transpose(2, 1, 0).reshape(M, T * BL),
             np.ones((1, T * BL), np.float32)], axis=0)
        m["frTT"] = cast("frTT", frTT)
        in_maps.append(m)
    return in_maps


def kernel(**inputs):
    mask = np.asarray(inputs["mask"])
    assert np.all(mask == 1.0), "kernel assumes all-ones mask"
    T = inputs["queries"].shape[1]

    in_maps = make_in_maps(inputs)
    nc = build_program(T)
    nc.compile()
    res = run_bass_kernel_spmd(nc, in_maps, list(range(NCORES)))
    out = np.zeros((B_FULL, T, S), np.float32)
    for c in range(NCORES):
        a = np.asarray(res.results[c]["alphas"], np.float32).reshape(T, BL, S)
        out[c * BL:(c + 1) * BL] = a.transpose(1, 0, 2)
    return out


if __name__ == "__main__":
    build_program(2)
    print("build ok")
